# revision 32
# baseline (speedup 1.0000x reference)
"""AdditiveAttention (Bahdanau) distributed Bass kernel for 8 TRN2 NeuronCores.

Computation (per batch b):
    qc[b,:]   = query[b] @ Wq + bq + bv                       # [512]  (HOST)
    z[b,s,:]  = value[b,s] @ Wv + qc[b]                       # pre-tanh
    score     = tanh(z) @ Wo          (+bo dropped: cancels in softmax)
    align     = softmax(score)        (no max-sub: |score| <= ~23)
    out[b,:]  = align @ value[b]

Sharding: data-parallel over batch, 4 batches per core, weights replicated.

v3 design (per core: B=4 batches, SEQ=4096, H=512):
  - HOST prep: qcombT (q-projection, transposed), Wo strip-padded to 32
    cols, selector matrices, and Wv pre-scaled x32 + packed to fp8e4 in
    the DoubleRow interleave h = 256*g + 2*p + i.
  - value: SWDGE cast-DMA f32->bf16 natural group tiles (context path),
    then DVE cast bf16->fp8 natural, then ONE xbar DMA-transpose per
    2-block pair of the fp8 data VIEWED AS bf16 pairs -> vT8 where each
    16-bit unit holds (h, h+1) adjacent fp8 values: exactly DoubleRow's
    moving-operand format. Transposed DMA bytes halve vs bf16 (8.4 MB).
  - main z matmuls: fp8 DoubleRow, K=256/pass -> 4 MMs of N=512 per
    (pair, hoc) accumulating a [128,1024] 2-bank PSUM tile (64 MMs/batch
    vs 128 bf16 MMs in v2).
  - tanh on ACT, N=1024 per instruction (vs 512), scale=1/32 undoes the
    Wv prescale, bias=qcombT adds the query projection.
  - scores: RAW (pre-exp) score strips accumulated in one PSUM bank per
    4-block group, 4-way col-tiled (M=32 strips so all 97 partitions are
    written), then ONE DVE copy -> s97, selector MMs transpose raw
    scores, and ONE exp per batch [128,32] with accum_out giving the
    softmax total for free. (v2 spent 23us/core on [1,512] exp calls.)
  - context: 32 accumulating MMs escT^T @ v_nat -> [1,512] rows, scale
    by 1/total, store via sync-queue DMA.
  - pipeline: slot calendar (b,pair,hoc); scores trail tanh by 2 slots;
    tails and next-batch value pipeline interleave into the z stream.
"""

import numpy as np

N_CORES = 8
BATCH_TOTAL = 32
B = BATCH_TOTAL // N_CORES  # batches per core
SEQ = 4096
H = 512
HC = H // 128   # 4 hidden chunks
NBLK = SEQ // 512   # 8 seq blocks per batch
NPAIR = NBLK // 2
WV_SCALE = 32.0

_cache = {}


def build_nc(b_per_core=B, seq=SEQ):
    from collections import deque

    import concourse.bass as bass
    import concourse.mybir as mybir
    import concourse.tile as tile
    from concourse import bacc
    from concourse.masks import make_identity

    f32 = mybir.dt.float32
    bf16 = mybir.dt.bfloat16
    fp8 = mybir.dt.float8e4
    AF = mybir.ActivationFunctionType
    DR = mybir.MatmulPerfMode.DoubleRow

    nblk = seq // 512
    npair = nblk // 2

    nc = bacc.Bacc("TRN2", target_bir_lowering=False, debug=False)

    val_d = nc.dram_tensor("value", [b_per_core, seq, H], f32, kind="ExternalInput").ap()
    wv8_d = nc.dram_tensor("wv8", [128, 2, 2, H], fp8, kind="ExternalInput").ap()
    qct_d = nc.dram_tensor("qcombT", [128, HC, b_per_core], f32, kind="ExternalInput").ap()
    wo32_d = nc.dram_tensor("wo32", [128, HC, 32], bf16, kind="ExternalInput").ap()
    sel_d = nc.dram_tensor("sel", [2, 98, 8], f32, kind="ExternalInput").ap()
    u32_d = nc.dram_tensor("u32", [b_per_core, 128, 4, 32], fp8, kind="ExternalInput").ap()
    out_d = nc.dram_tensor("out", [b_per_core, H], f32, kind="ExternalOutput").ap()

    # s = (g2*4 + blk)*512 + p*4 + j
    val_v = val_d.rearrange(
        "b (g blk p j) h -> b g p blk j h", g=2, blk=4, p=128, j=4
    )

    with tile.TileContext(nc) as tc:
        with (
            tc.tile_pool(name="weights", bufs=1) as wpool,
            tc.tile_pool(name="vnat", bufs=6) as vpool,
            tc.tile_pool(name="v8", bufs=3) as v8pool,
            tc.tile_pool(name="vt8", bufs=7) as tpool,
            tc.tile_pool(name="ht", bufs=9) as hpool,
            tc.tile_pool(name="small", bufs=6) as smpool,
            tc.tile_pool(name="psum_z", bufs=2, space="PSUM") as psz,
            tc.tile_pool(name="psum_tr", bufs=2, space="PSUM") as ptr,
            tc.tile_pool(name="psum_sc", bufs=1, space="PSUM") as pssc,
            tc.tile_pool(name="psum_tl", bufs=1, space="PSUM") as pstl,
        ):
            # ---- persistent SBUF residents ----
            Wv8_sb = wpool.tile([128, 2, 2, H], fp8)
            qcT = wpool.tile([128, HC, b_per_core], f32)
            Wo32_sb = wpool.tile([128, HC, 32], bf16)
            sel_sb = wpool.tile([98, 2, 8], f32)
            u32_sb = wpool.tile([128, b_per_core, 4, 32], fp8)
            ones128 = wpool.tile([128, 1], bf16)
            ones128f = wpool.tile([128, 1], f32)
            s97 = [wpool.tile([98, H], f32, name=f"s97_{g}") for g in range(2)]
            ctx97 = wpool.tile([97, H], bf16)
            warm = wpool.tile([128, H], bf16)
            prew_out = wpool.tile([1, 32], f32)
            ident = wpool.tile([128, 128], bf16)

            # ---- value pipeline ----
            vnats = {}   # (b, g2) -> bf16 natural tile [128, 4, 4, 512]
            v8s = {}     # (b, g2) -> fp8 natural tile  [128, 4, 2, 4, 256]
            vT8s = {}    # (b, p)  -> packed transpose  [128, 16, 128] bf16 units

            def load_group(b, g2):
                vt = vpool.tile([128, 4, 4, H], bf16, tag="vnat", name="vg")
                nc.gpsimd.dma_start(out=vt[:, 0:2], in_=val_v[b, g2, :, 0:2])
                nc.gpsimd.dma_start(out=vt[:, 2:4], in_=val_v[b, g2, :, 2:4])
                vnats[(b, g2)] = vt

            def cast_pair(b, pair):
                g2, q = pair // 2, pair % 2
                if (b, g2) not in v8s:
                    v8s[(b, g2)] = v8pool.tile(
                        [128, 4, 2, 4, 256], fp8, tag="v8", name="v8g")
                v8 = v8s[(b, g2)]
                vt = vnats[(b, g2)]
                for g in range(2):
                    nc.vector.tensor_copy(
                        v8[:, 2 * q:2 * q + 2, g],
                        vt[:, 2 * q:2 * q + 2, :, 256 * g:256 * (g + 1)])

            # PE-side transpose of the packed units: each [128,128] chunk of
            # the fp8-pair data (viewed as bf16 units) goes through matmul
            # transpose-mode into a bf16 PSUM bank (8 chunks per bank), then
            # one DVE copy lands it in vT8. No DMA-transposes at all: the
            # Tile framework serializes those against every other DMA (HW
            # deadlock guard), which lock-stepped loads and transposes into
            # a ~12us alternation in the v3a trace.
            def transpose_chunk(b, p, c, ps_tr):
                if (b, p) not in vT8s:
                    vT8s[(b, p)] = tpool.tile([128, 16, 128], bf16, tag="vt8",
                                              name="vt8")
                g2, q = p // 2, p % 2
                src = v8s[(b, g2)][:].bitcast(bf16).rearrange(
                    "p a b c d -> p (a b c d)")
                lo = q * 2048 + c * 128
                nc.tensor.matmul(ps_tr[:, c % 8, :], src[:, lo:lo + 128],
                                 ident[:], start=True, stop=True,
                                 is_transpose=True)

            def transpose_flush(b, p, c8, ps_tr):
                # copy chunks [c8, c8+8) of pair p from psum to vT8
                nc.vector.tensor_copy(vT8s[(b, p)][:, c8:c8 + 8, :], ps_tr[:])

            # transpose work queue: ("t", b, p, c) chunk transposes and
            # ("f", b, p, c8) psum->vT8 flushes, drained a few per slot.
            trans_q = deque()
            cur_ps = [None]

            def enqueue_transposes(b, p):
                for c in range(16):
                    trans_q.append(("t", b, p, c))
                    if c % 8 == 7:
                        trans_q.append(("f", b, p, c - 7))

            def drain_transposes(n):
                for _ in range(n):
                    if not trans_q:
                        return
                    it = trans_q.popleft()
                    if it[0] == "t":
                        _, b_, p_, c_ = it
                        if c_ % 8 == 0:
                            cur_ps[0] = ptr.tile([128, 8, 128], bf16,
                                                 tag="tr", name="pstr")
                        transpose_chunk(b_, p_, c_, cur_ps[0])
                    else:
                        _, b_, p_, c8 = it
                        transpose_flush(b_, p_, c8, cur_ps[0])

            # prologue — value-load triggers FIRST on the SWDGE queue (the
            # critical path; pure HBM loads now, so they free-run).
            load_group(0, 0)
            nc.gpsimd.memset(warm[:], 0.0)
            # ACT table prewarm: exp+tanh live in one set; load it while
            # the first value tiles stream in, not at first real tanh.
            nc.scalar.activation(prew_out[:], warm[0:1, 0:32], AF.Exp)
            nc.scalar.activation(prew_out[:], warm[0:1, 0:32], AF.Tanh)
            # weights ride the scalar (ACT) HWDGE queue: off the SWDGE
            # value-load critical path, done within a few us.
            nc.scalar.dma_start(out=Wv8_sb[:], in_=wv8_d)
            nc.scalar.dma_start(out=qcT[:], in_=qct_d)
            nc.scalar.dma_start(out=Wo32_sb[:], in_=wo32_d)
            nc.scalar.dma_start(out=sel_sb[:], in_=sel_d.rearrange("g p r -> p g r"))
            nc.scalar.dma_start(out=u32_sb[:], in_=u32_d.rearrange("b p k c -> p b k c"))
            load_group(0, 1)
            if b_per_core > 1:
                load_group(1, 0)
                load_group(1, 1)
            nc.gpsimd.memset(ones128[:], 1.0)
            nc.gpsimd.memset(ones128f[:], 1.0)
            nc.gpsimd.memset(ctx97[:], 0.0)
            make_identity(nc, ident[:])

            # PE warmup: fill the HAM activity window while value loads run.
            ps_warm = pstl.tile([128, H], f32, tag="tl", name="pswarm")
            for _ in range(32):
                nc.tensor.matmul(ps_warm[:], warm[:, 0:128], warm[:],
                                 start=True, stop=True)

            # batch-0 casts + transposes up front, PER PAIR so each pair's
            # psum->vT8 flush sits right behind its own cast in the DVE
            # queue (flushes behind all 8 data-gated casts cost 28us of PE
            # idle in the v3c trace).
            for p0_ in range(npair):
                cast_pair(0, p0_)
                enqueue_transposes(0, p0_)
                drain_transposes(len(trans_q))

            # ---- deferred-emission machinery ----
            pending_scores = deque()  # (b, g2, hoc, ready_slot)
            pending_tail = deque()    # (kind, b, g2, ready_slot)
            hTs = {}
            score_banks = {}
            tails = {}
            vt8fs = {}                # (b, p) -> fp8 AP view, for corrections

            def emit_scores(t):
                while pending_scores and pending_scores[0][3] <= t:
                    b_, g2, hoc, _ = pending_scores.popleft()
                    if hoc == 0:
                        score_banks[(b_, g2)] = pssc.tile([128, H], f32, tag="sc", name="ssum")
                    ssum = score_banks[(b_, g2)]
                    for pp in range(2):
                        hT = hTs.pop((b_, 2 * g2 + pp, hoc))
                        for bi in range(2):
                            row = 32 * (2 * pp + bi)
                            nc.tensor.matmul(
                                ssum[row:row + 32, :], Wo32_sb[:, hoc, :],
                                hT[:, 512 * bi:512 * (bi + 1)],
                                start=(hoc == 0), stop=False,
                                tile_position=(0, row),
                            )
                    if hoc == HC - 1:
                        # Wv-quantization correction rows: score strips left
                        # rows 32a+1 zero; accumulate corr = v8 . u there
                        # (1024x-scaled, in column 1 of an M=32 fp8 strip, so
                        # the MM shape matches the proven score strips); the
                        # selector subtracts it with coefficient 1/1024.
                        for g in range(2):
                            for i_ in range(2):
                                lhsT = u32_sb[:, b_, 2 * g + i_, :]
                                for a in range(4):
                                    pp, bi = a // 2, a % 2
                                    vt8f = vt8fs[(b_, 2 * g2 + pp)]
                                    c0 = 4 * (2 * bi + g)
                                    rhs = vt8f[:, c0:c0 + 4].rearrange(
                                        "p j (s i) -> p i (j s)", i=2)[:, i_, :]
                                    nc.tensor.matmul(
                                        ssum[32 * a:32 * a + 32, :], lhsT, rhs,
                                        start=False, stop=(g == 1 and i_ == 1),
                                        tile_position=(0, 32 * a),
                                        skip_group_check=True,
                                    )
                        pending_tail.append(("s97", b_, g2, t + 1))

            def emit_tail(t):
                if not (pending_tail and pending_tail[0][3] <= t):
                    return
                kind, b_, g2, _ = pending_tail.popleft()
                if kind == "s97":
                    ssum = score_banks.pop((b_, g2))
                    nc.vector.tensor_copy(s97[g2][:], ssum[0:98, :])
                    del vt8fs[(b_, 2 * g2)], vt8fs[(b_, 2 * g2 + 1)]
                    if g2 == 1:
                        pending_tail.append(("taila", b_, None, t + 1))
                elif kind == "taila":
                    pse = pstl.tile([128, HC, 8], f32, tag="tl", name="pse")
                    for j in range(HC):
                        for g2_ in range(2):
                            nc.tensor.matmul(
                                pse[:, j, :], s97[g2_][:, 128 * j:128 * (j + 1)],
                                sel_sb[:, g2_, :], start=(g2_ == 0), stop=(g2_ == 1),
                            )
                    escT = smpool.tile([128, HC, 8], bf16, tag="escT", name="escT")
                    eacc = smpool.tile([128, 1], f32, tag="eacc", name="eacc")
                    nc.scalar.activation(escT[:], pse[:], AF.Exp, accum_out=eacc[:])
                    tails[b_] = (escT, eacc)
                    pending_tail.append(("tailb", b_, None, t + 1))
                elif kind == "tailb":
                    escT, eacc = tails.pop(b_)
                    tot_ps = pstl.tile([1, 1], f32, tag="tl", name="totps")
                    nc.tensor.matmul(tot_ps[:], eacc[:], ones128f[:],
                                     start=True, stop=True)
                    rec = smpool.tile([1, 1], f32, tag="rec", name="rec")
                    nc.vector.reciprocal(rec[:], tot_ps[:])
                    ctx_ps = pstl.tile([128, H], f32, tag="tl", name="ctxps")
                    for blk in range(nblk):
                        for j in range(HC):
                            nc.tensor.matmul(
                                ctx_ps[32 * j:32 * j + 1, :],
                                escT[:, j, blk:blk + 1],
                                vnats[(b_, blk // 4)][:, blk % 4, j, :],
                                start=(blk == 0), stop=(blk == nblk - 1),
                                tile_position=(0, 32 * j),
                            )
                    for j in range(HC):
                        nc.vector.tensor_copy(
                            ctx97[32 * j:32 * j + 1, :], ctx_ps[32 * j:32 * j + 1, :],
                        )
                    cs_ps = pstl.tile([1, H], f32, tag="tl", name="csps")
                    nc.tensor.matmul(cs_ps[:], ones128[0:97, :], ctx97[:],
                                     start=True, stop=True)
                    outrow = smpool.tile([1, H], f32, tag="outrow", name="outrow")
                    nc.vector.tensor_scalar_mul(outrow[:], cs_ps[:], rec[:])
                    nc.sync.dma_start(out=out_d[b_:b_ + 1, :], in_=outrow[:])
                    del vnats[(b_, 0)], vnats[(b_, 1)]

            # value pipeline calendar during batch b: casts for b+1 (loads
            # landed a batch ago, so DVE never blocks), transposes for b+1
            # enqueued behind each cast and drained a few per slot, loads
            # for b+2 (free-running pure HBM stream).
            def cal_events(b, p, hoc):
                nb, nnb = b + 1, b + 2
                k = (p, hoc)
                if nb < b_per_core:
                    if k == (0, 0):
                        cast_pair(nb, 0)
                        enqueue_transposes(nb, 0)
                    elif k == (0, 1):
                        cast_pair(nb, 1)
                        enqueue_transposes(nb, 1)
                    elif k == (1, 0):
                        cast_pair(nb, 2)
                        enqueue_transposes(nb, 2)
                    elif k == (1, 1):
                        cast_pair(nb, 3)
                        enqueue_transposes(nb, 3)
                if nnb < b_per_core:
                    if k == (2, 0):
                        load_group(nnb, 0)
                    elif k == (3, 0):
                        load_group(nnb, 1)

            # ---------------- main pipeline ----------------
            t = 0
            for b in range(b_per_core):
                for p in range(npair):
                    vt8 = vT8s.pop((b, p))
                    vt8f = vt8[:].bitcast(fp8)   # [128, 16, 256]
                    vt8fs[(b, p)] = vt8f
                    for hoc in range(HC):
                        emit_scores(t)
                        emit_tail(t)
                        cal_events(b, p, hoc)
                        drain_transposes(6)
                        if b == 0:
                            # batch 0 is load-landing-paced (~5us per MB on
                            # the SWDGE read side); keep the HAM activity
                            # window filled so the real MMs stay at 2.4GHz.
                            for _ in range(3):
                                nc.tensor.matmul(ps_warm[:], warm[:, 0:128],
                                                 warm[:], start=True, stop=True)
                        zp = psz.tile([128, 1024], f32, tag="z", name="zp")
                        for g in range(2):
                            lhsT = Wv8_sb[:, g, :, 128 * hoc:128 * (hoc + 1)]
                            for bi in range(2):
                                c0 = 4 * (2 * bi + g)
                                rhs = vt8f[:, c0:c0 + 4].rearrange(
                                    "p j (s i) -> p i (j s)", i=2
                                )
                                nc.tensor.matmul(
                                    zp[:, 512 * bi:512 * (bi + 1)], lhsT, rhs,
                                    start=(g == 0), stop=(g == 1), perf_mode=DR,
                                )
                        hT = hpool.tile([128, 1024], bf16, tag="ht", name="hT")
                        nc.scalar.activation(
                            hT[:], zp[:], AF.Tanh,
                            bias=qcT[:, hoc, b:b + 1], scale=1.0 / WV_SCALE,
                        )
                        hTs[(b, p, hoc)] = hT
                        if p % 2 == 1:
                            pending_scores.append((b, p // 2, hoc, t + 2))
                        t += 1

            # drain
            while pending_scores or pending_tail:
                emit_scores(t)
                emit_tail(t)
                t += 1

    nc.compile()
    return nc


def make_in_maps(query, value, Wq, bq, Wv, bv, Wo, bo):
    """Host-side prep: shard + precompute small tensors. query [1,32,512]."""
    import ml_dtypes

    query = np.asarray(query, dtype=np.float32)
    value = np.asarray(value, dtype=np.float32)
    Wq = np.asarray(Wq, dtype=np.float32)
    bq = np.asarray(bq, dtype=np.float32)
    Wv = np.asarray(Wv, dtype=np.float32)
    bv = np.asarray(bv, dtype=np.float32)
    Wo = np.asarray(Wo, dtype=np.float32)

    # qcomb[b, h] = q[b] @ Wq + bq + bv
    qcomb = query[0] @ Wq + bq + bv                    # [32, 512]
    # Wv8[p, g, i, ho] = fp8(32 * Wv[256g + 2p + i, ho])
    wv_s = (WV_SCALE * Wv).reshape(2, 128, 2, H)       # [g, p, i, ho]
    wv8 = np.ascontiguousarray(
        wv_s.transpose(1, 0, 2, 3)).astype(ml_dtypes.float8_e4m3)  # [128,2,2,H]
    # Wo strips: [128, hc, 32], col 0 = Wo chunk, rest 0
    wo32 = np.zeros((128, HC, 32), np.float32)
    wo32[:, :, 0] = Wo[:, 0].reshape(HC, 128).T
    wo32 = wo32.astype(ml_dtypes.bfloat16)
    # selectors [2, 98, 8]: row 32a picks block score, row 32a+1 subtracts
    # the 1024x-scaled Wv-quantization correction
    sel = np.zeros((2, 98, 8), np.float32)
    for g2 in range(2):
        for a in range(4):
            sel[g2, 32 * a, g2 * 4 + a] = 1.0
            sel[g2, 32 * a + 1, g2 * 4 + a] = -1.0 / 1024.0

    # Wv-quantization score-correction weights: the fp8 weight error eW is
    # shared across all seq positions, creating a systematic score shift
    # ~ v_s . U_b with U_b[h] = sum_ho E[tanh'(z_bho)] * eW[h,ho] * Wo[ho].
    eW = wv8.astype(np.float32).transpose(1, 0, 2, 3).reshape(H, H) / WV_SCALE - Wv
    sig = np.sqrt((Wv ** 2).sum(0))                    # [512] std of v@Wv col
    gh_x, gh_w = np.polynomial.hermite_e.hermegauss(21)
    gh_w = gh_w / gh_w.sum()
    zz = qcomb[:, None, :] + sig[None, None, :] * gh_x[None, :, None]
    C = (gh_w[None, :, None] * (1.0 - np.tanh(zz) ** 2)).sum(1)  # [32, 512]
    Wo_b = wo32[:, :, 0].astype(np.float32).T.reshape(H)         # bf16-rounded Wo
    U = np.einsum('bk,hk,k->bh', C, eW, Wo_b)          # [32, 512]
    # u32[b, p, 2g+i, col] strip weights: col 1 = fp8(1024*U[b, 256g+2p+i])
    u8v = (1024.0 * U).reshape(32, 2, 128, 2).transpose(0, 2, 1, 3)  # [b,p,g,i]
    u32_full = np.zeros((32, 128, 4, 32), np.float32)
    u32_full[:, :, 0, 1] = u8v[:, :, 0, 0]
    u32_full[:, :, 1, 1] = u8v[:, :, 0, 1]
    u32_full[:, :, 2, 1] = u8v[:, :, 1, 0]
    u32_full[:, :, 3, 1] = u8v[:, :, 1, 1]
    u32_full = u32_full.astype(ml_dtypes.float8_e4m3)

    in_maps = []
    for i in range(N_CORES):
        sl = slice(B * i, B * (i + 1))
        qcT = np.ascontiguousarray(
            qcomb[sl].reshape(B, HC, 128).transpose(2, 1, 0))  # [128, HC, B]
        in_maps.append({
            "value": np.ascontiguousarray(value[sl]),
            "wv8": wv8,
            "qcombT": qcT,
            "wo32": wo32,
            "sel": sel,
            "u32": np.ascontiguousarray(u32_full[sl]),
        })
    return in_maps


def kernel(**inputs):
    from concourse.bass_utils import run_bass_kernel_spmd

    key = "full"
    if key not in _cache:
        _cache[key] = build_nc()
    nc = _cache[key]

    in_maps = make_in_maps(
        inputs["query"], inputs["value"], inputs["Wq"], inputs["bq"],
        inputs["Wv"], inputs["bv"], inputs["Wo"], inputs["bo"],
    )
    res = run_bass_kernel_spmd(nc, in_maps, core_ids=list(range(N_CORES)))
    out = np.concatenate([res.results[i]["out"] for i in range(N_CORES)], axis=0)
    return out[:, None, :].astype(np.float32)  # [32, 1, 512]
